# revision 24
# baseline (speedup 1.0000x reference)
"""Trainium2 kernel for nn_Linter_89000312307760 (segment_reduce).

Pipeline (zero host-side passes over the 128 MB feature tensor):
  host:   key = m*label + index per pixel (int32 -> f32, exact); counts via
          bincount. Features are shipped as raw f32 views.
  device: (8 cores, data-parallel: core = image*4 + quarter)
          - DMA feat [64, 65536] f32 in chunks (d-major, contiguous)
          - ACT downcasts the chunk to fp16; an XBAR transpose DMA rewrites
            it pixel-major [128, tiles, 64] (DMA engines, no compute cost)
          - one-hot per pixel-tile via tensor_scalar(iota == key) on DVE
          - segment sums via PSUM-accumulated matmuls: psum[64, 641] +=
            featT[128, 64]^T x onehot[128, 641] over all 512 tiles
          - one [64, 641] f32 output per core.
  host:   sum the 4 quarter outputs per image into segment means (tiny).
  device: (second dispatch, cores 0-3 = image 0, 4-7 = image 1, redundant
          within a group) L1 distance matrix A[i, j] = sum_d |m_i - m_j| via
          batched row-broadcast matmuls + fused abs-reduce on DVE, then
          pairsum = cmask^T A cmask with 42 small matmuls -> [5, 5] output.
  host:   huber + final -log scalar (microseconds).

The Bass module is input-independent: it is built once at import and reused
across calls; the first dispatch is warmed at import as well.
"""
import gc
import os
import sys
import time

import numpy as np

if "/opt/trn_rl_repo" not in sys.path:
    sys.path.insert(0, "/opt/trn_rl_repo")

try:  # persistent XLA executable cache: skips per-call wrapper recompiles
    import jax

    jax.config.update("jax_compilation_cache_dir", "/tmp/jax_comp_cache")
    jax.config.update("jax_persistent_cache_min_entry_size_bytes", -1)
    jax.config.update("jax_persistent_cache_min_compile_time_secs", 0.0)
except Exception:  # noqa: BLE001 — cache is an optimization only
    pass

import bass_rust
import concourse.bass as bass
import concourse.tile as tile
from concourse import mybir
from concourse.bass_utils import run_bass_kernel_spmd
from concourse.vector_clock import ScopedClock

# ---- problem constants (hardcoded per spec) ----
B, D, H, W = 2, 64, 512, 512
P = H * W                    # pixels per image
N_CLASSES = 5
IGNORE_LB = 255
S = N_CLASSES * 128 + 1      # 641 static segment capacity
N_CORES = 8
QUARTER = P // 4             # pixels per core chunk
N_TILES = QUARTER // 128     # 512 pixel-tiles per core
CHUNK_TILES = 32             # tiles per feat DMA chunk

LAST_RUN_WALL_S = None       # wall-clock of the device execute (set per call)


# ---------------------------------------------------------------- drain patch
def _patched_drain_and_barrier(self, tick_clock, wait_clock):
    # walrus CTRL ops encode only one sync wait; the stock kernel-tail drain
    # carries one wait per logical processor. Spread them over SP nops.
    nc = self.nc
    probe = nc.sync.nop(nofuse=True, hint="drain_wait_probe")
    wait_clock.add_sem_waits(probe.ins, ScopedClock({None: tick_clock.global_clock}))
    waits = list(probe.ins.sync_info.on_wait) if probe.ins.sync_info else []
    if len(waits) > 1:
        probe.ins.sync_info = bass_rust.SyncInfo(on_wait=waits[:1], on_update=[])
        for i, w in enumerate(waits[1:]):
            n = nc.sync.nop(nofuse=True, hint=f"drain_wait_{i}")
            n.ins.sync_info = bass_rust.SyncInfo(on_wait=[w], on_update=[])
    nc.sync.drain()
    nc.all_engine_barrier()
    assert self.sems is not None
    popped = nc._tile_sem_poison_stack.pop()
    assert popped is self._sem_poison
    nc.clear_and_free_semaphores(list(self.sems.allocated().values()))
    nc.all_engine_barrier()


tile.TileContext._drain_and_barrier = _patched_drain_and_barrier

_WSPLIT_N = 0


def _split_sync_waits(nc: bass.Bass):
    """walrus encodes at most one sync wait per instruction on this target;
    move extra waits onto same-engine nops inserted immediately before."""
    global _WSPLIT_N
    for f in nc.m.functions:
        for bb in f.blocks:
            out = []
            changed = False
            for ins in bb.instructions:
                si = ins.sync_info
                if si is not None and si.on_wait and len(si.on_wait) > 1:
                    changed = True
                    waits = list(si.on_wait)
                    for w in waits[:-1]:
                        _WSPLIT_N += 1
                        out.append(
                            mybir.InstNoOp(
                                name=f"WSPLIT-{_WSPLIT_N}",
                                engine=ins.engine,
                                bass_nofuse=True,
                                sync_info=mybir.SyncInfo(on_wait=[w], on_update=[]),
                            )
                        )
                    ins.sync_info = mybir.SyncInfo(
                        on_wait=[waits[-1]], on_update=list(si.on_update)
                    )
                out.append(ins)
            if changed:
                bb.instructions = out


# ---------------------------------------------------------------- device part
def build_device_kernel(
    n_tiles: int = N_TILES,
    chunk_tiles: int = CHUNK_TILES,
    fc_bufs: int = 3,
    ft_bufs: int = 3,
    oh_bufs: int = 8,
) -> bass.Bass:
    nc = bass.Bass("TRN2")
    f16 = mybir.dt.float16
    f32 = mybir.dt.float32

    i32 = mybir.dt.int32
    feat_d = nc.declare_dram_parameter("feat", [64, n_tiles * 128], f32, isOutput=False)
    lab_d = nc.declare_dram_parameter("lab", [128, n_tiles], i32, isOutput=False)
    idx_d = nc.declare_dram_parameter("idx", [128, n_tiles], i32, isOutput=False)
    m_d = nc.declare_dram_parameter("mval", [128, 1], f32, isOutput=False)
    iota_d = nc.declare_dram_parameter("iota", [128, S], f16, isOutput=False)
    out_d = nc.declare_dram_parameter("out", [65, S], f32, isOutput=True)

    n_chunks = (n_tiles + chunk_tiles - 1) // chunk_tiles

    with tile.TileContext(nc) as tc:
        with (
            tc.tile_pool(name="const", bufs=1) as const_tp,
            tc.tile_pool(name="fc", bufs=fc_bufs) as fc_tp,
            tc.tile_pool(name="fc16", bufs=fc_bufs) as fc16_tp,
            tc.tile_pool(name="ft", bufs=ft_bufs) as ft_tp,
            tc.tile_pool(name="oh", bufs=oh_bufs) as oh_tp,
            tc.tile_pool(name="o", bufs=1) as out_tp,
            tc.tile_pool(name="ps", bufs=1, space="PSUM") as ps_tp,
        ):
            iota_sb = const_tp.tile([128, S], f16)
            nc.sync.dma_start(out=iota_sb[:], in_=iota_d[:])
            lab_sb = const_tp.tile([128, n_tiles], i32)
            nc.sync.dma_start(out=lab_sb[:], in_=lab_d[:])
            idx_sb = const_tp.tile([128, n_tiles], i32)
            nc.sync.dma_start(out=idx_sb[:], in_=idx_d[:])
            m_sb = const_tp.tile([128, 1], f32)
            nc.sync.dma_start(out=m_sb[:], in_=m_d[:])
            # key = m*label + index, on the otherwise idle Pool engine.
            # IGNORE pixels (label 255) get keys >= S and match no one-hot
            # column, which reproduces the reference's segment-0 exclusion.
            keys_mul = const_tp.tile([128, n_tiles], f32)
            nc.gpsimd.tensor_scalar(
                out=keys_mul[:],
                in0=lab_sb[:],
                scalar1=m_sb[:, 0:1],
                scalar2=None,
                op0=mybir.AluOpType.mult,
            )
            keys_sb = const_tp.tile([128, n_tiles], f32)
            nc.gpsimd.tensor_tensor(
                out=keys_sb[:],
                in0=keys_mul[:],
                in1=idx_sb[:],
                op=mybir.AluOpType.add,
            )

            # row 64 of the stationary is all-ones: PSUM row 64 accumulates
            # per-segment pixel counts for free
            psA = ps_tp.tile([65, 512], f32, space="PSUM")
            psB = ps_tp.tile([65, S - 512], f32, space="PSUM")

            for c in range(n_chunks):
                t0 = c * chunk_tiles
                t1 = min(t0 + chunk_tiles, n_tiles)
                w = (t1 - t0) * 128
                # d-major f32 chunk in; downcast on ACT; XBAR-transpose to
                # pixel-major fp16 [128, tiles, 64] (DMA engines, no compute)
                fc32 = fc_tp.tile([64, w], f32, tag="fc")
                nc.sync.dma_start(out=fc32[:], in_=feat_d[:, t0 * 128 : t1 * 128])
                fc16 = fc16_tp.tile([80, w], f16, tag="fc16")
                nc.scalar.activation(
                    out=fc16[0:64], in_=fc32[:],
                    func=mybir.ActivationFunctionType.Copy,
                )
                nc.gpsimd.memset(fc16[64:80], 0.0)
                nc.gpsimd.memset(fc16[64:65], 1.0)
                ft = ft_tp.tile([128, t1 - t0, 80], f16, tag="ft")
                nc.sync.dma_start_transpose(out=ft[:], in_=fc16[:])
                for t in range(t0, t1):
                    lt = t - t0
                    oh = oh_tp.tile([128, S], f16, tag="oh")
                    nc.vector.tensor_scalar(
                        out=oh[:],
                        in0=iota_sb[:],
                        scalar1=keys_sb[:, t : t + 1],
                        scalar2=None,
                        op0=mybir.AluOpType.is_equal,
                    )
                    nc.tensor.matmul(
                        out=psA[:],
                        lhsT=ft[:, lt, 0:65],
                        rhs=oh[:, 0:512],
                        start=(t == 0),
                        stop=(t == n_tiles - 1),
                    )
                    nc.tensor.matmul(
                        out=psB[:],
                        lhsT=ft[:, lt, 0:65],
                        rhs=oh[:, 512:S],
                        start=(t == 0),
                        stop=(t == n_tiles - 1),
                    )

            out_sb = out_tp.tile([65, S], f32)
            nc.scalar.activation(
                out=out_sb[:, 0:512], in_=psA[:],
                func=mybir.ActivationFunctionType.Copy,
            )
            nc.scalar.activation(
                out=out_sb[:, 512:S], in_=psB[:],
                func=mybir.ActivationFunctionType.Copy,
            )
            nc.sync.dma_start(out=out_d[:], in_=out_sb[:])

    _split_sync_waits(nc)
    return nc


NSEG = 640                   # segments 1..640 (slot = segment-1; seg 0 is never valid)
NG = NSEG // 128
GRP = 8                      # columns per rows/rep matmul group


def build_phase2_kernel(diff_bufs=4, pool_share=0):
    """Pairwise class-pair loss on device: A = L1 distance matrix over segment
    means, pair = cmask^T A cmask. Means arrive d-major f32 [64, 768] only;
    the segment-partition fp16 layout is built on-device with 6 PE transposes.
    Row broadcast per 8-column group: PE transpose extracts the 8 rows, a DMA
    flattens them to one partition, one matmul replicates across partitions."""
    nc = bass.Bass("TRN2")
    f16 = mybir.dt.float16
    f32 = mybir.dt.float32
    meansd_d = nc.declare_dram_parameter("meansd", [64, NSEG], f32, isOutput=False)
    ident_d = nc.declare_dram_parameter("ident", [64, 64], f32, isOutput=False)
    ident128_d = nc.declare_dram_parameter("ident128", [128, 128], f32, isOutput=False)
    cmask_d = nc.declare_dram_parameter("cmask", [128, NG * N_CLASSES], f32, isOutput=False)
    pair_d = nc.declare_dram_parameter("pair", [N_CLASSES, N_CLASSES], f32, isOutput=True)

    C = N_CLASSES
    n_grp = (NSEG + GRP - 1) // GRP

    with tile.TileContext(nc) as tc:
        with (
            tc.tile_pool(name="const", bufs=1) as const_tp,
            tc.tile_pool(name="A", bufs=1) as a_tp,
            tc.tile_pool(name="mrow", bufs=3) as mrow_tp,
            tc.tile_pool(name="diff", bufs=diff_bufs) as diff_tp,
            tc.tile_pool(name="o", bufs=1) as out_tp,
            tc.tile_pool(name="rows", bufs=2, space="PSUM") as rows_tp,
            tc.tile_pool(name="rep", bufs=2, space="PSUM") as rep_tp,
            tc.tile_pool(name="sym", bufs=1, space="PSUM") as sym_tp,
            tc.tile_pool(name="ps2", bufs=1, space="PSUM") as ps2_tp,
        ):
            meansd_sb = const_tp.tile([64, NSEG], f32)
            nc.sync.dma_start(out=meansd_sb[:], in_=meansd_d[:])
            ident_sb = const_tp.tile([64, 64], f32)
            nc.sync.dma_start(out=ident_sb[:], in_=ident_d[:])
            ident128_sb = const_tp.tile([128, 128], f32)
            nc.sync.dma_start(out=ident128_sb[:], in_=ident128_d[:])
            cm_sb = const_tp.tile([128, NG, C], f32)
            nc.sync.dma_start(out=cm_sb[:], in_=cmask_d[:])
            ones_sb = const_tp.tile([1, 128], f32)
            nc.vector.memset(ones_sb[:], 1.0)

            # segment-partition fp16 means from the d-major layout
            means_sb = const_tp.tile([128, NG, 64], f16)
            for g in range(NG):
                mt_ps = rows_tp.tile([128, 64], f32, space="PSUM", tag="rows")
                nc.tensor.transpose(
                    out=mt_ps[:],
                    in_=meansd_sb[:, g * 128 : (g + 1) * 128],
                    identity=ident_sb[:],
                )
                nc.scalar.activation(
                    out=means_sb[:, g, :], in_=mt_ps[:],
                    func=mybir.ActivationFunctionType.Copy,
                )

            A_sb = a_tp.tile([128, NG, NSEG], f32)
            nc.gpsimd.memset(A_sb[:], 0.0)

            for g in range(n_grp):
                j0 = g * GRP
                k = min(j0 + GRP, NSEG) - j0
                # extract rows j0..j0+k to partitions 0..k, then flatten to
                # partition 0 with a DMA (engines stay free)
                rows_ps = rows_tp.tile([128, 64], f32, space="PSUM", tag="rows")
                nc.tensor.transpose(
                    out=rows_ps[:k],
                    in_=meansd_sb[:, j0 : j0 + k],
                    identity=ident_sb[:],
                )
                rows_sb = mrow_tp.tile([GRP, 64], f32, tag="rows_sb")
                nc.scalar.activation(
                    out=rows_sb[:k], in_=rows_ps[:k],
                    func=mybir.ActivationFunctionType.Copy,
                )
                mrow = mrow_tp.tile([1, GRP * 64], f32, tag="mrow")
                nc.sync.dma_start(
                    out=mrow[:, : k * 64].rearrange("p (j d) -> p j d", d=64),
                    in_=rows_sb[:k],
                )
                # replicate the k rows across all 128 partitions in one matmul
                rep_ps = rep_tp.tile([128, GRP * 64], f32, space="PSUM", tag="rep")
                nc.tensor.matmul(
                    out=rep_ps[:, : k * 64],
                    lhsT=ones_sb[:],
                    rhs=mrow[:, : k * 64],
                    start=True,
                    stop=True,
                )
                rep_sb = mrow_tp.tile([128, GRP * 64], f16, tag="repsb")
                nc.scalar.activation(
                    out=rep_sb[:, : k * 64], in_=rep_ps[:, : k * 64],
                    func=mybir.ActivationFunctionType.Copy,
                )
                # Lower triangle only: columns in block gj need row-blocks
                # gj..NG-1 (A is symmetric; upper blocks come from transposes)
                gj = j0 // 128
                nb = NG - gj
                eng = nc.gpsimd if (g % 100) < pool_share else nc.vector
                diff = diff_tp.tile([128, GRP, NG, 64], f16, tag="diff")
                reprs = rep_sb[:, : k * 64].rearrange("p (j d) -> p j d", d=64)
                eng.tensor_tensor(
                    out=diff[:, :k, :nb],
                    in0=means_sb[:, None, gj:, :].broadcast_to((128, k, nb, 64)),
                    in1=reprs[:, :, None, :].broadcast_to((128, k, nb, 64)),
                    op=mybir.AluOpType.subtract,
                )
                nc.vector.tensor_reduce(
                    out=A_sb[:, gj:, j0 : j0 + k].rearrange("p g j -> p j g"),
                    in_=diff[:, :k, :nb],
                    axis=mybir.AxisListType.X,
                    op=mybir.AluOpType.add,
                    apply_absolute_value=True,
                )

            # fill upper-triangle blocks: A[gr-rows, gc-cols] = A[gc-rows, gr-cols]^T
            for gr in range(NG):
                for gc in range(gr + 1, NG):
                    sym_ps = sym_tp.tile([128, 128], f32, space="PSUM", tag="symt")
                    nc.tensor.transpose(
                        out=sym_ps[:],
                        in_=A_sb[:, gc, gr * 128 : (gr + 1) * 128],
                        identity=ident128_sb[:],
                    )
                    nc.scalar.activation(
                        out=A_sb[:, gr, gc * 128 : (gc + 1) * 128], in_=sym_ps[:],
                        func=mybir.ActivationFunctionType.Copy,
                    )

            # B[i, c] = sum_j A[j, i] * cm[j, c]  (A symmetric)
            ps2 = ps2_tp.tile([128, NG, C], f32, space="PSUM")
            for gi in range(NG):
                for gj in range(NG):
                    nc.tensor.matmul(
                        out=ps2[:, gi, :],
                        lhsT=A_sb[:, gj, gi * 128 : (gi + 1) * 128],
                        rhs=cm_sb[:, gj, :],
                        start=(gj == 0),
                        stop=(gj == NG - 1),
                    )
            b_sb = out_tp.tile([128, NG, C], f32)
            nc.scalar.activation(
                out=b_sb[:], in_=ps2[:], func=mybir.ActivationFunctionType.Copy
            )
            # pair[c1, c2] = sum_i cm[i, c1] * B[i, c2]
            ps3 = ps2_tp.tile([C, C], f32, space="PSUM", tag="ps3")
            for gi in range(NG):
                nc.tensor.matmul(
                    out=ps3[:],
                    lhsT=cm_sb[:, gi, :],
                    rhs=b_sb[:, gi, :],
                    start=(gi == 0),
                    stop=(gi == NG - 1),
                )
            pair_sb = out_tp.tile([C, C], f32)
            nc.scalar.activation(
                out=pair_sb[:], in_=ps3[:], func=mybir.ActivationFunctionType.Copy
            )
            nc.sync.dma_start(out=pair_d[:], in_=pair_sb[:])

    _split_sync_waits(nc)
    return nc


_NC = None
_NC2 = None


def _get_nc() -> bass.Bass:
    global _NC
    if _NC is None:
        _NC = build_device_kernel()
    return _NC


def _get_nc2() -> bass.Bass:
    global _NC2
    if _NC2 is None:
        _NC2 = build_phase2_kernel()
    return _NC2


# ------------------------------------------------------------------ host part
_IOTA = None
_IDENT = None
_IDENT128 = None


def _get_ident() -> np.ndarray:
    global _IDENT
    if _IDENT is None:
        _IDENT = np.eye(64, dtype=np.float32)
    return _IDENT


def _get_ident128() -> np.ndarray:
    global _IDENT128
    if _IDENT128 is None:
        _IDENT128 = np.eye(128, dtype=np.float32)
    return _IDENT128


def _get_iota() -> np.ndarray:
    global _IOTA
    if _IOTA is None:
        _IOTA = np.ascontiguousarray(
            np.broadcast_to(np.arange(S, dtype=np.float16), (128, S))
        )
    return _IOTA


def _as_i32(a):
    """int64 -> int32 without a host pass: little-endian low words as a
    strided view (values are small non-negatives). The copy to contiguous
    happens inside the dispatch's input concat."""
    if a.dtype == np.int64 and a.flags.c_contiguous:
        return a.reshape(B, P).view(np.int32)[:, ::2]
    return np.ascontiguousarray(a).reshape(B, P).astype(np.int32)


def _host_prep(feature_out, labels, indexes):
    """Builds per-core in_maps (all views, no host data passes)."""
    lab = _as_i32(np.asarray(labels))
    idx = _as_i32(np.asarray(indexes))
    m = np.asarray(indexes).reshape(B, P).max(axis=1)     # per-image max index

    feat = np.asarray(feature_out, dtype=np.float32).reshape(B, D, P)

    iota = _get_iota()
    in_maps = []
    for core in range(N_CORES):
        b, q = divmod(core, 4)
        lo = q * QUARTER
        in_maps.append(
            {
                "feat": feat[b][:, lo : lo + QUARTER],
                "lab": lab[b, lo : lo + QUARTER].reshape(N_TILES, 128).T,
                "idx": idx[b, lo : lo + QUARTER].reshape(N_TILES, 128).T,
                "mval": np.full((128, 1), float(m[b]), np.float32),
                "iota": iota,
            }
        )
    return in_maps, m


def _phase2_inputs(sums_b, counts_b, m_b):
    """Device phase2 inputs for one image from [64, S] f32 quarter-summed
    segment sums. Returns (meansT16 [128, NG*64] f16, meansd [64, NSEG] f32,
    cmT [128, NG*C] f32, n_c [C])."""
    cnt = counts_b.astype(np.float32)
    valid = cnt >= 2.0
    valid[0] = False
    inv = np.where(valid, 1.0 / np.maximum(cnt, 1.0), 0.0).astype(np.float32)

    # slot j holds segment j+1 (segment 0 is never valid)
    meansd = np.ascontiguousarray(sums_b[:, 1:] * inv[None, 1:])  # [D, NSEG]

    seg = np.arange(1, NSEG + 1)
    cls = np.clip(np.ceil(seg / float(m_b)) - 1.0, 0, N_CLASSES - 1).astype(np.int64)
    cm = np.zeros((NSEG, N_CLASSES), np.float32)
    cm[np.arange(NSEG), cls] = 1.0
    cm[~valid[1:]] = 0.0
    cmT = np.ascontiguousarray(
        cm.reshape(NG, 128, N_CLASSES).transpose(1, 0, 2).reshape(128, NG * N_CLASSES)
    )
    n_c = cm.sum(axis=0).astype(np.float64)
    return meansd, cmT, n_c


def _finalize(pair, n_c):
    """Huber class-pair loss from the device [5, 5] pairsum (sum over D of
    L1 distances; reference uses the mean over D)."""
    npair = np.outer(n_c, n_c)
    ret = (pair.astype(np.float64) / float(D)) / np.maximum(npair, 1.0)
    h = np.where(ret < 1.0, 0.5 * ret * ret, ret - 0.5)
    tri = np.triu(np.ones((N_CLASSES, N_CLASSES)), k=1)
    pv = tri * (npair > 0.0)
    return float((h * pv).sum()), float(pv.sum())


def kernel(feature_out, labels, indexes):
    global LAST_RUN_WALL_S
    gc_was_enabled = gc.isenabled()
    gc.disable()
    try:
        return _kernel_impl(feature_out, labels, indexes)
    finally:
        if gc_was_enabled:
            gc.enable()


def _kernel_impl(feature_out, labels, indexes):
    global LAST_RUN_WALL_S
    in_maps, m = _host_prep(feature_out, labels, indexes)

    nc = _get_nc()
    t0 = time.monotonic()
    res = run_bass_kernel_spmd(nc, in_maps, core_ids=list(range(N_CORES)))
    LAST_RUN_WALL_S = time.monotonic() - t0

    ident = _get_ident()
    in_maps2 = []
    n_cs = []
    for b in range(B):
        sums65 = (
            res.results[4 * b + 0]["out"]
            + res.results[4 * b + 1]["out"]
            + res.results[4 * b + 2]["out"]
            + res.results[4 * b + 3]["out"]
        )                                                 # [65, S]: sums + counts
        meansd, cmT, n_c = _phase2_inputs(sums65[:D], sums65[D], int(m[b]))
        n_cs.append(n_c)
        in_maps2.append(
            {"meansd": meansd, "ident": ident, "ident128": _get_ident128(), "cmask": cmT}
        )
    # cores 0-3 -> image 0, cores 4-7 -> image 1 (redundant within a group)
    in_maps2 = [in_maps2[c // 4] for c in range(N_CORES)]

    t0 = time.monotonic()
    res2 = run_bass_kernel_spmd(_get_nc2(), in_maps2, core_ids=list(range(N_CORES)))
    LAST_RUN_WALL_S += time.monotonic() - t0

    tot_s = tot_c = 0.0
    for b in range(B):
        s_img, c_img = _finalize(res2.results[4 * b]["pair"], n_cs[b])
        tot_s += s_img
        tot_c += c_img

    mean_h = tot_s / max(tot_c, 1.0)
    mean_h = max(mean_h, 1e-12)
    out = -np.log(mean_h / float(B)) if tot_c > 0 else 0.0
    return np.array([out], dtype=np.float32)


# Build the module at import; warm the dispatch path (jit trace + NEFF cache +
# program load) so the first timed kernel() call pays only transfer + exec.
if os.environ.get("KERNEL_NO_WARM", "") != "1":
    try:
        _get_nc()
        _zero_maps = [
            {
                "feat": np.zeros((64, QUARTER), np.float32),
                "lab": np.zeros((128, N_TILES), np.int32),
                "idx": np.ones((128, N_TILES), np.int32),
                "mval": np.ones((128, 1), np.float32),
                "iota": _get_iota(),
            }
            for _ in range(N_CORES)
        ]
        run_bass_kernel_spmd(_get_nc(), _zero_maps, core_ids=list(range(N_CORES)))
        del _zero_maps
        _zero_maps2 = [
            {
                "meansd": np.zeros((D, NSEG), np.float32),
                "ident": _get_ident(),
                "ident128": _get_ident128(),
                "cmask": np.zeros((128, NG * N_CLASSES), np.float32),
            }
            for _ in range(N_CORES)
        ]
        run_bass_kernel_spmd(_get_nc2(), _zero_maps2, core_ids=list(range(N_CORES)))
        del _zero_maps2
    except Exception as _e:  # noqa: BLE001 — warmup is best-effort only
        sys.stderr.write(f"kernel warmup skipped: {_e}\n")
    try:  # warm the host numpy paths too (allocator, ufunc dispatch, sort)
        _f = np.zeros((B, D, H, W), np.float32)
        _l = np.zeros((B, H, W), np.int64)
        _i = np.ones((B, H, W), np.int64)
        _host_prep(_f, _l, _i)
        _s = np.arange(D * S, dtype=np.float32).reshape(D, S)
        _c = np.full(S, 3, np.int64)
        _md, _cmT, _nc_ = _phase2_inputs(_s, _c, 128)
        _finalize(np.ones((N_CLASSES, N_CLASSES), np.float32), _nc_)
        del _f, _l, _i, _s, _c, _md, _cmT, _nc_
    except Exception as _e:  # noqa: BLE001
        sys.stderr.write(f"host warmup skipped: {_e}\n")
